# revision 1
# baseline (speedup 1.0000x reference)
"""Trainium2 Bass kernel for nn_BatchRelationalModule (gnn_message_passing).

Reference computation (per batch b of 32):
  x = [imgfeat(128) | coord] per position l in 0..143            # [L, 129]
  gi = x @ W1[:129]   (indexed by j);  gjb = x @ W1[129:] + b1   # [L, 64]
  A[:, (i,j)] = lrelu(gi[j] + gjb[i])                            # [64, L*L]
  P = W2.T @ A + b2;  s = sum_{i,j} lrelu(P)                     # [64]
  out = lrelu(lrelu(s @ Wp + bp) @ Wo + bo)                      # [64]

Sharding: data-parallel over batch, 4 batches per core (2 groups of 2
batches stacked on SBUF partitions: rows 0-63 = even batch features,
rows 64-127 = odd batch features).

Per-core device pipeline (per 2-batch group):
  PE  : gi/gjb prep matmuls (x @ W, fp16) into PSUM halves
  DVE : custom fused op  Z = lrelu(gi_bcast + gjb_bcast)  (one pass,
        broadcast via 0-stride access patterns, fp32 in / fp16 out)
        + accum_out = rowsum(Z) in fp32
  PE  : W2.T @ Z as bf16 hi+lo accumulating matmul pairs (~16-bit
        effective weight mantissa; bf16 keeps the fp32 exponent range so
        the lo part never denormal-flushes), col-tiled so both batches
        run concurrently and PSUM packs [128, fd]
  ACT : relu(0.99*(P + b2)) with per-partition bias + fused accum_out
  final: sum lrelu(P+b2) = 0.01*(W2.T @ rowsum(Z) + Npair*b2) + accum(relu),
  assembled with tiny per-batch matmuls (identity-matmul moves the
  odd-batch partition halves), then the small MLP on PE/DVE (fp32).

All constants arrive in 4 packed DMA transfers (per-transfer overhead
~0.6us dominates small loads).
"""

import os
import sys

import numpy as np

for _p in ("/opt/trn_rl_repo",):
    if os.path.isdir(_p) and _p not in sys.path:
        sys.path.insert(0, _p)

import operator

import concourse.bass as bass
import concourse.tile as tile
from concourse import bacc, mybir
from concourse.bass import _add_dep_helper

B, C = 32, 128
L = 144
HID = 64
NCORES = 8
BPC = 4  # batches per core
NPAIR = L * L  # 20736
SLOPE = 0.01
LIN_COEF = SLOPE          # weight of the exact linear term
RELU_COEF = 1.0 - SLOPE   # weight of the relu-sum term
PSUM_FD = 2048
SCH = [8, 16, 24, 32, 32, 32]  # i-chunk ramp, capped so consumers never wait
# Per-group ACT tile plans. Group 0 starts with small tiles so the first
# ACT fires as soon as the first Z chunk lands; group 1 is already
# pipelined and uses full tiles.
PLANS = [[512, 1024] + [2048] * 9 + [768], [2048] * 10 + [256]]
assert all(sum(p) == NPAIR for p in PLANS)

# fp32 constant pack column map
_C_GA2 = 0          # [128, 144]
_C_GB2 = 144        # [128, 144]
_C_B2C = 288        # [128, 1]
_C_W2S = 289        # [128, 64] (0.01*W2 duplicated into both halves)
_C_I64 = 353        # [128, 64] (identity duplicated into both halves)
_C_WP = 417         # [64, 64]
_C_WO = 481         # [64, 64]
_C_BP4 = 545        # [64, 4]
_C_BO4 = 549        # [64, 4]
_C_C2 = 553         # [64, 1] (0.01 * NPAIR * b2 as a per-partition column)
_C32_COLS = 554

_cache: dict = {}


def _register_op():
    """Register the fused lrelu(Src0 + Src1) custom DVE op at runtime."""
    from concourse import dve_ops
    from concourse.dve_spec import Spec, Src0, Src1, C0, maxx, lower, _has_src1
    from concourse.dve_uop import DveOpSpec

    name = "LRELU_ADD_ANT"
    if name in dve_ops._SUB_OPCODE_FOR_NAME:
        return next(o for o in dve_ops.OPS if o.name == name)

    def _ref(in0, in1, s0, s1, imm2):
        z = np.asarray(in0, np.float32) + np.asarray(in1, np.float32)
        s0v = s0 if isinstance(s0, float) else np.asarray(s0, np.float32)
        out = np.maximum(z, z * s0v)
        acc = out.reshape(out.shape[0], -1).sum(axis=-1, keepdims=True)
        return out, acc.astype(np.float32)

    _z = Src0 + Src1
    spec = Spec(body=maxx(_z, _z * C0), accum=operator.add, reference=_ref)
    op = dve_ops.DveOp(name, spec, subdim=False, uops_sha={})
    dve_ops.OPS.append(op)
    row = dve_ops._CUSTOM_DVE_ROW_BASE + len(dve_ops.OPS) - 1
    assert row < 0x20
    dve_ops._SUB_OPCODE_FOR_NAME[name] = row
    dve_ops.CUSTOM_DVE_SPECS[name] = spec
    for ver in ("v3", "v4"):
        try:
            uops = lower(spec, ver=ver)
            sha = DveOpSpec(
                name=name, opcode=row, uops=uops, rd1_en=_has_src1(spec)
            ).sha(ver)
            op.uops_sha[ver] = sha
        except Exception:
            pass
    return op


def _register_post_op():
    """relu(Src0 * C1 + C0) with accum_out — DVE version of the ACT post pass."""
    from concourse import dve_ops
    from concourse.dve_spec import Spec, Src0, C0, C1, relu, lower, _has_src1
    from concourse.dve_uop import DveOpSpec

    name = "RELU_BIAS_ACC_ANT"
    if name in dve_ops._SUB_OPCODE_FOR_NAME:
        return next(o for o in dve_ops.OPS if o.name == name)

    def _ref(in0, in1, s0, s1, imm2):
        x = np.asarray(in0, np.float32) * s1 + (
            s0 if isinstance(s0, float) else np.asarray(s0, np.float32)
        )
        out = np.maximum(x, 0.0)
        acc = out.reshape(out.shape[0], -1).sum(axis=-1, keepdims=True)
        return out, acc.astype(np.float32)

    spec = Spec(body=relu(Src0 * C1 + C0), accum=operator.add, reference=_ref)
    op = dve_ops.DveOp(name, spec, subdim=False, uops_sha={})
    dve_ops.OPS.append(op)
    row = dve_ops._CUSTOM_DVE_ROW_BASE + len(dve_ops.OPS) - 1
    assert row < 0x20
    dve_ops._SUB_OPCODE_FOR_NAME[name] = row
    dve_ops.CUSTOM_DVE_SPECS[name] = spec
    for ver in ("v3", "v4"):
        try:
            uops = lower(spec, ver=ver)
            sha = DveOpSpec(
                name=name, opcode=row, uops=uops, rd1_en=_has_src1(spec)
            ).sha(ver)
            op.uops_sha[ver] = sha
        except Exception:
            pass
    return op


def _bcast_in0(ap, S):
    """[128, L] -> [128, S, L] repeating the whole tile S times (0-stride)."""
    return bass.AP(ap.tensor, ap.offset, [ap.ap[0], [0, S], *ap.ap[1:]])


def _bcast_in1(ap, n_inner):
    """[128, S] -> [128, S, n_inner] repeating each column (0-stride inner)."""
    return bass.AP(ap.tensor, ap.offset, [*ap.ap, [0, n_inner]])


def build_nc():
    """Build the Bass module (identical for every core)."""
    LRELU = _register_op()
    RELUB = _register_post_op()
    nc = bacc.Bacc(trn_type="TRN2")
    f32 = mybir.dt.float32
    f16 = mybir.dt.float16
    bf16 = mybir.dt.bfloat16
    AF = mybir.ActivationFunctionType

    d_xall = nc.dram_tensor("xall", [BPC, 128, L], f16, kind="ExternalInput")
    d_pk16 = nc.dram_tensor("pk16", [128, 2 * HID], f16, kind="ExternalInput")
    d_pkbf = nc.dram_tensor("pkbf", [128, 2 * HID], bf16, kind="ExternalInput")
    d_pk32 = nc.dram_tensor("pk32", [128, _C32_COLS], f32, kind="ExternalInput")
    d_out = nc.dram_tensor("out", [HID, BPC], f32, kind="ExternalOutput")

    with tile.TileContext(nc) as tc:
        with (
            tc.tile_pool(name="const", bufs=1) as cp,
            tc.tile_pool(name="g", bufs=2) as gp,
            tc.tile_pool(name="zl", bufs=4) as zlp,
            tc.tile_pool(name="trash", bufs=2) as trp,
            tc.tile_pool(name="small", bufs=1) as smp,
            tc.tile_pool(name="psum", bufs=2, space=bass.MemorySpace.PSUM) as pp,
        ):
            xall = cp.tile([128, BPC * L], f16, tag="xall")
            # pack batches along the free dim; group 0's two batches first so
            # prep can start while the rest of the constants stream in
            def xperm(ap):
                return bass.AP(ap.tensor, ap.offset, [ap.ap[1], ap.ap[0], ap.ap[2]])

            nc.sync.dma_start(xall[:, 0 : 2 * L], xperm(d_xall[0:2]))
            pk16 = cp.tile([128, 2 * HID], f16, tag="pk16")
            nc.sync.dma_start(pk16[:], d_pk16[:])
            pk32 = cp.tile([128, _C32_COLS], f32, tag="pk32")
            nc.sync.dma_start(pk32[:, 0 : _C_B2C + 1], d_pk32[:, 0 : _C_B2C + 1])
            nc.sync.dma_start(xall[:, 2 * L : 4 * L], xperm(d_xall[2:4]))
            nc.sync.dma_start(
                pk32[:, _C_B2C + 1 :], d_pk32[:, _C_B2C + 1 :]
            )
            pkbf = cp.tile([128, 2 * HID], bf16, tag="pkbf")
            nc.sync.dma_start(pkbf[:], d_pkbf[:])

            t_xf = [xall[:, L * b : L * (b + 1)] for b in range(BPC)]
            t_wa = pk16[:, 0:HID]
            t_wb = pk16[:, HID : 2 * HID]
            t_whi = pkbf[:, 0:HID]
            t_wlo = pkbf[:, HID : 2 * HID]
            t_ga2 = pk32[:, _C_GA2 : _C_GA2 + L]
            t_gb2 = pk32[:, _C_GB2 : _C_GB2 + L]
            t_b2c = pk32[:, _C_B2C : _C_B2C + 1]
            t_w2s = pk32[:, _C_W2S : _C_W2S + HID]
            t_i64 = pk32[:, _C_I64 : _C_I64 + HID]
            t_wp = pk32[0:HID, _C_WP : _C_WP + HID]
            t_wo = pk32[0:HID, _C_WO : _C_WO + HID]
            t_bp4 = pk32[0:HID, _C_BP4 : _C_BP4 + BPC]
            t_bo4 = pk32[0:HID, _C_BO4 : _C_BO4 + BPC]
            t_c2 = pk32[0:HID, _C_C2 : _C_C2 + 1]

            accz = smp.tile([128, 16], f32, tag="accz")
            absc = smp.tile([128, 32], f32, tag="absc")
            zsumg = smp.tile([128, 2], f32, tag="zsumg")
            asumg = smp.tile([128, 2], f32, tag="asumg")

            nc.gpsimd.memset(absc[:, 27:28], 0.0)

            # ---- prep: gi2 / gjb2 for both groups (PSUM halves per batch) --
            gi2s, gjb2s = [], []
            for g in range(2):
                ps_gi = pp.tile([128, L], f32, tag="mm")
                nc.tensor.matmul(ps_gi[0:64, :], t_wa, t_xf[2 * g])
                nc.tensor.matmul(ps_gi[64:128, :], t_wa, t_xf[2 * g + 1])
                gi2 = gp.tile([128, L], f32, tag="gi2")
                nc.vector.tensor_add(gi2[:], ps_gi[:], t_ga2)
                ps_gj = pp.tile([128, L], f32, tag="mm")
                nc.tensor.matmul(ps_gj[0:64, :], t_wb, t_xf[2 * g])
                nc.tensor.matmul(ps_gj[64:128, :], t_wb, t_xf[2 * g + 1])
                gjb2 = gp.tile([128, L], f32, tag="gjb2")
                nc.vector.tensor_add(gjb2[:], ps_gj[:], t_gb2)
                gi2s.append(gi2)
                gjb2s.append(gjb2)

            # ---- main: per group, fused-lrelu Z tiles -> matmuls -> ACT ----
            for g in range(2):
                gi2, gjb2 = gi2s[g], gjb2s[g]
                segs = []  # (tile, start_col, n_cols)
                i0 = 0
                zi_insts = []
                for ci, S in enumerate(SCH):
                    zt = zlp.tile([128, S * L], bf16, tag="zl")
                    in0 = _bcast_in0(gi2[:, 0:L], S)
                    in1 = _bcast_in1(gjb2[:, i0 : i0 + S], L)
                    zi = nc.vector._custom_dve(
                        LRELU,
                        out=zt[:],
                        in0=in0,
                        in1=in1,
                        s0=SLOPE,
                        accum_out=accz[:, 8 * g + ci : 8 * g + ci + 1],
                    )
                    zi_insts.append(zi)
                    segs.append((zt, i0 * L, S * L))
                    i0 += S

                def seg_for(c):
                    for (zt, s0_, n_) in segs:
                        if s0_ <= c < s0_ + n_:
                            return zt, c - s0_, s0_ + n_ - c
                    raise AssertionError(c)

                c = 0
                act_insts = []
                for ti, fd in enumerate(PLANS[g]):
                    ps = pp.tile([128, PSUM_FD], f32, tag="mm")
                    pcol = 0
                    while pcol < fd:
                        zt, zoff, zleft = seg_for(c)
                        n = min(512 - (pcol % 512), zleft, fd - pcol)
                        for h in range(2):  # batch half: partitions 64h..
                            r = slice(64 * h, 64 * h + 64)
                            nc.tensor.matmul(
                                ps[r, pcol : pcol + n],
                                t_whi[r, :],
                                zt[r, zoff : zoff + n],
                                start=True,
                                stop=False,
                            )
                            nc.tensor.matmul(
                                ps[r, pcol : pcol + n],
                                t_wlo[r, :],
                                zt[r, zoff : zoff + n],
                                start=False,
                                stop=True,
                            )
                        c += n
                        pcol += n
                    tr = trp.tile([128, PSUM_FD], f16, tag="tr")
                    if g == 1 and ti in (8, 9):
                        # split the post pass: ACT and DVE each take half of
                        # this PSUM tile so the pipeline tail drains ~2x faster
                        hf = fd // 2
                        ai = nc.scalar.activation(
                            tr[:, 0:hf],
                            ps[:, 0:hf],
                            AF.Relu,
                            bias=t_b2c,
                            scale=RELU_COEF,
                            accum_out=absc[:, 16 + ti : 16 + ti + 1],
                        )
                        act_insts.append(ai)
                        di = nc.vector._custom_dve(
                            RELUB,
                            out=tr[:, hf:fd],
                            in0=ps[:, hf:fd],
                            s0=t_b2c,
                            s1=RELU_COEF,
                            accum_out=absc[:, 28 + (ti - 8) : 29 + (ti - 8)],
                        )
                        act_insts.append(di)
                    else:
                        ai = nc.scalar.activation(
                            tr[:, 0:fd],
                            ps[:, 0:fd],
                            AF.Relu,
                            bias=t_b2c,
                            scale=RELU_COEF,
                            accum_out=absc[:, 16 * g + ti : 16 * g + ti + 1],
                        )
                        act_insts.append(ai)
                assert c == NPAIR

                # per-group: fold the per-chunk accumulators. The reduces
                # must wait for the accum_out (second-output) writes, which
                # Tile's dependency tracker does not see — add explicit edges.
                rz = nc.vector.tensor_reduce(
                    zsumg[:, g : g + 1],
                    accz[:, 8 * g : 8 * g + len(SCH)],
                    axis=mybir.AxisListType.X,
                    op=mybir.AluOpType.add,
                )
                for zi in zi_insts:
                    _add_dep_helper(rz.ins, zi.ins, sync=True, reason="accz accum_out")
                ra = nc.vector.tensor_reduce(
                    asumg[:, g : g + 1],
                    absc[:, 16 * g : 16 * g + (len(PLANS[g]) if g == 0 else 14)],
                    axis=mybir.AxisListType.X,
                    op=mybir.AluOpType.add,
                )
                for ai in act_insts:
                    _add_dep_helper(ra.ins, ai.ins, sync=True, reason="absc accum_out")

            # ---- tail: s = 0.01*(W2.T zsum + N b2) + relu-accum, tiny MLP --
            zsum_all = smp.tile([HID, BPC], f32, tag="zsum_all")
            asum_all = smp.tile([HID, BPC], f32, tag="asum_all")
            for b in range(BPC):
                g, h = divmod(b, 2)
                r = slice(64 * h, 64 * h + 64)
                if h == 0:
                    nc.vector.tensor_copy(
                        zsum_all[0:64, b : b + 1], zsumg[r, g : g + 1]
                    )
                    nc.vector.tensor_copy(
                        asum_all[0:64, b : b + 1], asumg[r, g : g + 1]
                    )
                else:
                    nc.sync.dma_start(zsum_all[0:64, b : b + 1], zsumg[r, g : g + 1])
                    nc.sync.dma_start(asum_all[0:64, b : b + 1], asumg[r, g : g + 1])
            lz = pp.tile([HID, BPC], f32, tag="mm")
            nc.tensor.matmul(lz[:], t_w2s[0:HID, :], zsum_all[:])
            s_all = smp.tile([HID, BPC], f32, tag="s_all")
            # s = lz + asum (c2 is folded into bp on the host); s0=1.0 makes
            # the lrelu op an exact add: max(z, z*1) = z
            nc.vector._custom_dve(
                LRELU, out=s_all[:], in0=lz[:], in1=asum_all[:], s0=1.0
            )

            p1 = pp.tile([HID, BPC], f32, tag="mm")
            nc.tensor.matmul(p1[:], t_wp, s_all[:])
            h1 = smp.tile([HID, BPC], f32, tag="h1")
            nc.vector._custom_dve(LRELU, out=h1[:], in0=p1[:], in1=t_bp4, s0=SLOPE)
            p2 = pp.tile([HID, BPC], f32, tag="mm")
            nc.tensor.matmul(p2[:], t_wo, h1[:])
            fin = smp.tile([HID, BPC], f32, tag="fin")
            nc.vector._custom_dve(LRELU, out=fin[:], in0=p2[:], in1=t_bo4, s0=SLOPE)
            nc.sync.dma_start(d_out[:], fin[:])

    nc.compile()
    return nc


def host_prep(inputs):
    """Host-side weight preprocessing -> shared input map + per-core xall."""
    x_img = np.asarray(inputs["x_img"], np.float32)
    W1 = np.asarray(inputs["W1"], np.float32)
    b1 = np.asarray(inputs["b1"], np.float32)
    W2 = np.asarray(inputs["W2"], np.float32)
    b2 = np.asarray(inputs["b2"], np.float32)
    Wp = np.asarray(inputs["Wp"], np.float32)
    bp = np.asarray(inputs["bp"], np.float32)
    Wo = np.asarray(inputs["Wo"], np.float32)
    bo = np.asarray(inputs["bo"], np.float32)
    import ml_dtypes

    BF = ml_dtypes.bfloat16

    x = x_img.reshape(B, C, L)  # [b, c, l]
    coords = np.arange(L, dtype=np.float32)
    GaT = (coords[:, None] * W1[C][None, :]).T  # [64, 144]
    GbT = (coords[:, None] * W1[C + 1 + C][None, :] + b1[None, :]).T
    W2hi = W2.astype(BF)
    W2lo = (W2 - W2hi.astype(np.float32)).astype(BF)

    pk16 = np.zeros((128, 2 * HID), np.float16)
    pk16[:, 0:HID] = W1[:C].astype(np.float16)
    pk16[:, HID : 2 * HID] = W1[C + 1 : C + 1 + C].astype(np.float16)

    pkbf = np.zeros((128, 2 * HID), BF)
    pkbf[0:64, 0:HID] = W2hi
    pkbf[64:128, 0:HID] = W2hi
    pkbf[0:64, HID:] = W2lo
    pkbf[64:128, HID:] = W2lo

    pk32 = np.zeros((128, _C32_COLS), np.float32)
    pk32[:, _C_GA2 : _C_GA2 + L] = np.concatenate([GaT, GaT], 0)
    pk32[:, _C_GB2 : _C_GB2 + L] = np.concatenate([GbT, GbT], 0)
    pk32[:, _C_B2C] = np.tile(RELU_COEF * b2, 2)
    pk32[0:64, _C_W2S : _C_W2S + HID] = LIN_COEF * W2
    pk32[64:128, _C_W2S : _C_W2S + HID] = LIN_COEF * W2
    eye = np.eye(HID, dtype=np.float32)
    pk32[0:64, _C_I64 : _C_I64 + HID] = eye
    pk32[64:128, _C_I64 : _C_I64 + HID] = eye
    pk32[0:HID, _C_WP : _C_WP + HID] = Wp
    pk32[0:HID, _C_WO : _C_WO + HID] = Wo
    bp_eff = bp + (LIN_COEF * NPAIR * b2) @ Wp  # fold the c2 constant into bp
    pk32[0:HID, _C_BP4 : _C_BP4 + BPC] = np.repeat(bp_eff[:, None], BPC, axis=1)
    pk32[0:HID, _C_BO4 : _C_BO4 + BPC] = np.repeat(bo[:, None], BPC, axis=1)
    pk32[0:HID, _C_C2] = LIN_COEF * NPAIR * b2

    base = {
        "pk16": np.ascontiguousarray(pk16),
        "pkbf": np.ascontiguousarray(pkbf),
        "pk32": np.ascontiguousarray(pk32),
    }
    in_maps = []
    for k in range(NCORES):
        m = dict(base)
        m["xall"] = np.ascontiguousarray(
            x[BPC * k : BPC * (k + 1)].astype(np.float16)
        )
        in_maps.append(m)
    return in_maps


def kernel(**inputs) -> np.ndarray:
    from concourse.bass_utils import run_bass_kernel_spmd

    if "nc" not in _cache:
        _cache["nc"] = build_nc()
    nc = _cache["nc"]
    in_maps = host_prep(inputs)
    res = run_bass_kernel_spmd(nc, in_maps, core_ids=list(range(NCORES)))
    out = np.concatenate([r["out"].T for r in res.results], axis=0)  # [32, 64]
    return np.ascontiguousarray(out, np.float32)



# revision 5
# speedup vs baseline: 1.0829x; 1.0829x over previous
"""Trainium2 Bass kernel for nn_BatchRelationalModule (gnn_message_passing).

Reference computation (per batch b of 32):
  x = [imgfeat(128) | coord] per position l in 0..143            # [L, 129]
  gi = x @ W1[:129]   (indexed by j);  gjb = x @ W1[129:] + b1   # [L, 64]
  Z[:, (i,j)] = lrelu(gi[j] + gjb[i])                            # [64, L*L]
  P = W2.T @ Z + b2;  s = sum_{i,j} lrelu(P)                     # [64]
  out = lrelu(lrelu(s @ Wp + bp) @ Wo + bo)                      # [64]

Sharding: data-parallel over batch, 4 batches per core, 2 groups of 2
batches stacked on SBUF partitions (rows 0-63 / 64-127).

Key points of this implementation:
  - gi/gjb are tiny per-batch tensors; the host computes them (numpy) and
    ships gi pre-duplicated (each column repeated 32/16 times) so the
    device Z-gen op runs with packed innermost access patterns.
  - Z-gen runs on DVE as a custom fused op lrelu(in0+in1) with a
    hand-written 2X_1PORT uop program (elem pairs via SRC_*_HI lanes) --
    2 elem/lane/cycle, ~0.54 ns/col vs 1.06 at the stock 1x. in1 is the
    gjb broadcast [p,[0,J],[1,SI]]; inner runs must be >=32B (SI>=16).
  - W2 is applied as ONE fp16 matmul per 512-col chunk with a [128,128]
    block-diagonal stationary (both batch halves in one pass, K=128).
  - The pair reduction uses ACT Lrelu directly: accum_out of
    lrelu(psum + b2) summed per partition. No 0.01/0.99 relu fold, no
    sum(Z) accumulators. A slice of tiles runs on DVE (custom single-src
    lrelu(x+b2) op with accum) to balance the two engines.
"""

import os
import sys

import numpy as np

for _p in ("/opt/trn_rl_repo",):
    if os.path.isdir(_p) and _p not in sys.path:
        sys.path.insert(0, _p)

import operator

import concourse.bass as bass
import concourse.tile as tile
from concourse import bacc, bass_isa, mybir
from concourse.bass import _add_dep_helper

B, C = 32, 128
L = 144
HID = 64
NCORES = 8
BPC = 4  # batches per core
SLOPE = 0.01
PSUM_FD = 2048
# i-chunks per group: 4x32 + 1x16 (144 total); in1 inner run = SI*2 bytes
ICHUNKS = [32, 32, 32, 32, 16]
# j-splits of the first chunk so matmuls can start early
J_SPLIT0 = [36, 36, 72]
# PSUM tile plans per group (cols each, sum = 20736)
PLANS = [[512, 1024] + [2048] * 9 + [768], [2048] * 10 + [256]]
NPAIR = L * L
assert all(sum(p) == NPAIR for p in PLANS)
# which plan tiles the DVE handles (rest go to ACT)
DVE_TILES = [{4}, {3, 9, 10}]
# extra Z columns emitted ahead of a tile's P-pass (DVE just-in-time slack)
Z_SLACK = 2048

# fp32 constant pack column map
_C_B2C = 0          # [128, 1]
_C_WP = 1           # [64, 64]
_C_WO = 65          # [64, 64]
_C_BP4 = 129        # [64, 4]
_C_BO4 = 133        # [64, 4]
_C32_COLS = 137

_cache: dict = {}


def _register_lrelu2x():
    """Fused Z = lrelu(in0 + in1), body-only, with a hand-written
    2X_1PORT uop program (two fp16 elements per lane-cycle)."""
    from concourse import dve_ops
    from concourse.dve_spec import Spec, Src0, Src1, C0, maxx, lower
    from concourse.dve_uop import (
        AluInp,
        AluOp,
        DelayInp,
        DveOpSpec,
        InpSel,
        OutPath,
        OutSel,
        Trigger,
        UopConfig,
    )

    name = "LRELU2X_ANT"
    if name in dve_ops._SUB_OPCODE_FOR_NAME:
        return next(o for o in dve_ops.OPS if o.name == name)

    def _ref(in0, in1, s0, s1, imm2):
        a = np.asarray(in0, np.float32).reshape(in0.shape[0], -1)
        b = np.asarray(in1, np.float32).reshape(in1.shape[0], -1)
        z = a + b
        s0v = s0 if isinstance(s0, float) else np.asarray(s0, np.float32)
        return np.maximum(z, z * s0v)

    _z = Src0 + Src1
    spec = Spec(body=maxx(_z, _z * C0), reference=_ref)
    op = dve_ops.DveOp(name, spec, subdim=False, uops_sha={})
    dve_ops.OPS.append(op)
    row = dve_ops._CUSTOM_DVE_ROW_BASE + len(dve_ops.OPS) - 1
    assert row < 0x20
    dve_ops._SUB_OPCODE_FOR_NAME[name] = row
    dve_ops.CUSTOM_DVE_SPECS[name] = spec

    uops1x = lower(spec, ver="v3")
    assert len(uops1x) == 1

    # 2X_1PORT: elem0 through blocks 0-2, elem1 (SRC_*_HI) through 3-5,
    # elem0's result rides delay chain 0 to the write mux.
    u = UopConfig()
    u.enable_input(InpSel.SRC_0, 1)      # a0 -> PD0 at blk0
    u.enable_input(InpSel.SRC_1, 2)      # b0 -> PD1
    u.enable_input(InpSel.CONST_0, 3)    # c0 -> PD2
    u.enable_input(InpSel.SRC_0_HI, 4)   # a1 -> PD3
    u.enable_input(InpSel.SRC_1_HI, 5)   # b1 -> PD4
    u.require_inp0 = 1
    u.require_inp1 = 1
    u.trigger = (Trigger.SRC_TENSOR_DONE, Trigger.NONE, Trigger.NONE)
    u.next_uop = (0, 0, 0)
    u.enable_output(OutSel.DELAY_0, OutPath.WR0_LO)   # r0
    u.enable_output(OutSel.ALU_OUT, OutPath.WR0_HI)   # r1
    dp = u.datapath_config
    dp[0].enable_alu(AluOp.ADD, AluInp.PREV_DELAY_0, AluInp.PREV_DELAY_1)
    dp[0].pass_through_delay(2, 3, 4)
    dp[1].enable_alu(AluOp.MULTIPLY, AluInp.PREV_ALU_OUT, AluInp.PREV_DELAY_2)
    dp[1].enable_delay_from_src(DelayInp.PREV_ALU_OUT, 0)
    dp[1].pass_through_delay(2, 3, 4)
    dp[2].enable_alu(AluOp.MAX, AluInp.PREV_DELAY_0, AluInp.PREV_ALU_OUT)
    dp[2].pass_through_delay(2, 3, 4)
    dp[3].enable_alu(AluOp.ADD, AluInp.PREV_DELAY_3, AluInp.PREV_DELAY_4)
    dp[3].enable_delay_from_src(DelayInp.PREV_ALU_OUT, 0)
    dp[3].pass_through_delay(2)
    dp[4].enable_alu(AluOp.MULTIPLY, AluInp.PREV_ALU_OUT, AluInp.PREV_DELAY_2)
    dp[4].enable_delay_from_src(DelayInp.PREV_ALU_OUT, 1)
    dp[4].pass_through_delay(0)
    dp[5].enable_alu(AluOp.MAX, AluInp.PREV_DELAY_1, AluInp.PREV_ALU_OUT)
    dp[5].pass_through_delay(0)
    dp[6].pass_through_alu()
    dp[6].pass_through_delay(0)
    dp[7].pass_through_alu()
    dp[7].pass_through_delay(0)

    full = DveOpSpec(
        name=name, opcode=row, uops=uops1x, uops_2x=[u], rd1_en=True, perf_max=1
    )
    full.validate("v3")
    op.uops_sha["v3"] = full.sha("v3")
    dve_ops._COMPILE_CACHE[(name, "v3")] = full
    return op


def _register_lrelu_bias_acc():
    """Single-source op for the DVE share of the pair reduction:
    out = lrelu(in0 + s0),  accum_out = rowsum(out).  s0 = per-partition b2."""
    from concourse import dve_ops
    from concourse.dve_spec import Spec, Src0, C0, C1, maxx, lower, _has_src1
    from concourse.dve_uop import DveOpSpec

    name = "LRELU_BIAS_ACC_ANT"
    if name in dve_ops._SUB_OPCODE_FOR_NAME:
        return next(o for o in dve_ops.OPS if o.name == name)

    def _ref(in0, in1, s0, s1, imm2):
        x = np.asarray(in0, np.float32)
        s0v = s0 if isinstance(s0, float) else np.asarray(s0, np.float32)
        s1v = s1 if isinstance(s1, float) else np.asarray(s1, np.float32)
        y = x + s0v
        out = np.maximum(y, y * s1v)
        acc = out.reshape(out.shape[0], -1).sum(axis=-1, keepdims=True)
        return out, acc.astype(np.float32)

    _y = Src0 + C0
    spec = Spec(body=maxx(_y, _y * C1), accum=operator.add, reference=_ref)
    op = dve_ops.DveOp(name, spec, subdim=False, uops_sha={})
    dve_ops.OPS.append(op)
    row = dve_ops._CUSTOM_DVE_ROW_BASE + len(dve_ops.OPS) - 1
    assert row < 0x20
    dve_ops._SUB_OPCODE_FOR_NAME[name] = row
    dve_ops.CUSTOM_DVE_SPECS[name] = spec
    full = DveOpSpec(
        name=name,
        opcode=row,
        uops=lower(spec, ver="v3"),
        rd1_en=_has_src1(spec),
    )
    op.uops_sha["v3"] = full.sha("v3")
    dve_ops._COMPILE_CACHE[(name, "v3")] = full
    return op


def _emit_z(eng, op, *, out, in0, in1, s0):
    """Emit the Z-gen custom op with perf_max=1 (2X_1PORT enabled)."""
    nc_bass = eng.bass
    if op.name not in nc_bass.m.ant_custom_dve_ops:
        nc_bass.m.ant_custom_dve_ops = sorted(
            {*nc_bass.m.ant_custom_dve_ops, op.name}
        )
    from concourse.dve_ops import get_dve_sub_opcode

    shape = bass_isa.CustomDveShape.STT
    isa_opcode = nc_bass.isa.Opcode[
        f"NEURON_ISA_TPB_OPCODE_CUSTOM_DVE_ANT_{shape.slot()}"
    ].value
    ins = [
        eng.lower_ap(in0, for_isa=True, opt=True),
        eng.lower_ap(in1, for_isa=True, opt=True),
        mybir.ImmediateValue(dtype=mybir.dt.float32, value=float(s0)),
        mybir.ImmediateValue(dtype=mybir.dt.float32, value=0.0),
    ]
    outs = [eng.lower_ap(out, for_isa=True, opt=True)]
    return eng.add_instruction(
        bass_isa.InstCustomDveAnt(
            name=nc_bass.get_next_instruction_name(),
            op_name=op.name,
            rd1_en=True,
            subdim=0,
            imm2=0.0,
            shape=shape,
            row=get_dve_sub_opcode(op.name),
            isa_opcode=isa_opcode,
            ins=ins,
            outs=outs,
            perf_max=1,
        )
    )


def build_nc():
    LRELU2X = _register_lrelu2x()
    LRELUB = _register_lrelu_bias_acc()
    nc = bacc.Bacc(trn_type="TRN2")
    f32 = mybir.dt.float32
    f16 = mybir.dt.float16
    AF = mybir.ActivationFunctionType

    d_gid32 = nc.dram_tensor("gid32", [2, 128, 32 * L], f16, kind="ExternalInput")
    d_gid16 = nc.dram_tensor("gid16", [2, 128, 16 * L], f16, kind="ExternalInput")
    d_gjb = nc.dram_tensor("gjb", [2, 128, L], f16, kind="ExternalInput")
    d_w2d = nc.dram_tensor("w2d", [128, 128], f16, kind="ExternalInput")
    d_c32 = nc.dram_tensor("c32", [128, _C32_COLS], f32, kind="ExternalInput")
    d_out = nc.dram_tensor("out", [HID, BPC], f32, kind="ExternalOutput")

    with tile.TileContext(nc) as tc:
        with (
            tc.tile_pool(name="const", bufs=1) as cp,
            tc.tile_pool(name="z32", bufs=5) as zp,
            tc.tile_pool(name="z16", bufs=2) as zp16,
            tc.tile_pool(name="trash", bufs=3) as trp,
            tc.tile_pool(name="small", bufs=1) as smp,
            tc.tile_pool(name="psum", bufs=2, space=bass.MemorySpace.PSUM) as pp,
        ):
            # ---- constants / inputs -------------------------------------
            gjb_t = [cp.tile([128, L], f16, tag=f"gjb{g}", name=f"gjb{g}") for g in range(2)]
            gid32_t = [cp.tile([128, 32 * L], f16, tag=f"g32_{g}", name=f"g32_{g}") for g in range(2)]
            gid16_t = [cp.tile([128, 16 * L], f16, tag=f"g16_{g}", name=f"g16_{g}") for g in range(2)]
            w2d = cp.tile([128, 128], f16, tag="w2d")
            c32 = cp.tile([128, _C32_COLS], f32, tag="c32")
            warm = cp.tile([128, 16], f16, tag="warm")
            warm2 = cp.tile([128, 16], f16, tag="warm2")

            nc.gpsimd.memset(warm[:], 0.25)
            nc.sync.dma_start(gjb_t[0][:], d_gjb[0])
            nc.sync.dma_start(c32[:], d_c32[:])
            nc.sync.dma_start(w2d[:], d_w2d[:])
            # group-0 gi duplicates, split so the first Z instr starts early
            JQ = [0, 36 * 32, 72 * 32, 144 * 32]
            for a, b2_ in zip(JQ[:-1], JQ[1:]):
                nc.sync.dma_start(gid32_t[0][:, a:b2_], d_gid32[0][:, a:b2_])
            nc.sync.dma_start(gid16_t[0][:], d_gid16[0])
            nc.sync.dma_start(gjb_t[1][:], d_gjb[1])
            for a, b2_ in zip(JQ[:-1], JQ[1:]):
                nc.sync.dma_start(gid32_t[1][:, a:b2_], d_gid32[1][:, a:b2_])
            nc.sync.dma_start(gid16_t[1][:], d_gid16[1])

            t_b2c = c32[:, _C_B2C : _C_B2C + 1]
            t_wp = c32[0:HID, _C_WP : _C_WP + HID]
            t_wo = c32[0:HID, _C_WO : _C_WO + HID]
            t_bp4 = c32[0:HID, _C_BP4 : _C_BP4 + BPC]
            t_bo4 = c32[0:HID, _C_BO4 : _C_BO4 + BPC]

            # early ACT table load for Lrelu (off the critical path)
            nc.scalar.activation(warm2[:], warm[:], AF.Lrelu, bias=0.0,
                                 scale=1.0, alpha=SLOPE)

            accs = smp.tile([128, 32], f32, tag="accs")  # 16 cols per group
            asumg = smp.tile([128, 2], f32, tag="asumg")

            # ---- main pipeline ------------------------------------------
            # Per group: walk the PSUM tile plan; before each tile, emit
            # just enough Z chunks (plus Z_SLACK) to cover its columns.
            # This puts the DVE-share pair-reduction instrs into the DVE
            # queue right where their inputs are already available.
            red_insts = [[], []]
            for g in range(2):
                chunks = []       # (tile, ncols) in col order
                cum_z = [0]       # emitted Z cols
                chunk_iter = iter(range(len(ICHUNKS)))

                def emit_next_chunk():
                    ci = next(chunk_iter)
                    si = ICHUNKS[ci]
                    ncols = si * L
                    pool = zp if si == 32 else zp16
                    zt = pool.tile(
                        [128, ncols], f16, tag="z" if si == 32 else "zz",
                        name=f"zt{g}_{ci}",
                    )
                    gid = gid32_t[g] if si == 32 else gid16_t[g]
                    i0 = sum(ICHUNKS[:ci])
                    jsplits = J_SPLIT0 if (g == 0 and ci == 0) else [L]
                    j0 = 0
                    for js in jsplits:
                        a = gjb_t[g][:]
                        in1 = bass.AP(
                            a.tensor, a.offset + i0, [a.ap[0], [0, js], [1, si]]
                        )
                        _emit_z(
                            nc.vector, LRELU2X,
                            out=zt[:, j0 * si : (j0 + js) * si],
                            in0=gid[:, j0 * si : (j0 + js) * si],
                            in1=in1, s0=SLOPE,
                        )
                        j0 += js
                    chunks.append((zt, ncols))
                    cum_z[0] += ncols

                def seg_for(c):
                    off = 0
                    for (zt, n) in chunks:
                        if c < off + n:
                            return zt, c - off, off + n - c
                        off += n
                    raise AssertionError(c)

                c = 0
                for ti, fd in enumerate(PLANS[g]):
                    while cum_z[0] < min(c + fd + Z_SLACK, NPAIR):
                        emit_next_chunk()
                    ps = pp.tile([128, PSUM_FD], f32, tag="mm")
                    pcol = 0
                    while pcol < fd:
                        zt, zoff, zleft = seg_for(c)
                        n = min(512 - (pcol % 512), zleft, fd - pcol)
                        nc.tensor.matmul(
                            ps[:, pcol : pcol + n],
                            w2d[:],
                            zt[:, zoff : zoff + n],
                            start=True,
                            stop=True,
                        )
                        c += n
                        pcol += n
                    tr = trp.tile([128, PSUM_FD], f16, tag="tr")
                    acc_ap = accs[:, 16 * g + ti : 16 * g + ti + 1]
                    if ti in DVE_TILES[g]:
                        ri = nc.vector._custom_dve(
                            LRELUB,
                            out=tr[:, 0:fd],
                            in0=ps[:, 0:fd],
                            s0=t_b2c,
                            s1=SLOPE,
                            accum_out=acc_ap,
                        )
                    else:
                        ri = nc.scalar.activation(
                            tr[:, 0:fd],
                            ps[:, 0:fd],
                            AF.Lrelu,
                            bias=t_b2c,
                            scale=1.0,
                            alpha=SLOPE,
                            accum_out=acc_ap,
                        )
                    red_insts[g].append(ri)
                assert c == NPAIR and cum_z[0] == NPAIR

            # ---- per-group accumulator fold -----------------------------
            for g in range(2):
                ra = nc.vector.tensor_reduce(
                    asumg[:, g : g + 1],
                    accs[:, 16 * g : 16 * g + len(PLANS[g])],
                    axis=mybir.AxisListType.X,
                    op=mybir.AluOpType.add,
                )
                for ri in red_insts[g]:
                    _add_dep_helper(ra.ins, ri.ins, sync=True, reason="accum_out")

            # ---- tail: tiny MLP ----------------------------------------
            s_all = smp.tile([HID, BPC], f32, tag="s_all")
            for b in range(BPC):
                g, h = divmod(b, 2)
                src = asumg[64 * h : 64 * h + 64, g : g + 1]
                if h == 0:
                    nc.vector.tensor_copy(s_all[0:HID, b : b + 1], src)
                else:
                    nc.sync.dma_start(s_all[0:HID, b : b + 1], src)
            p1 = pp.tile([HID, BPC], f32, tag="mm")
            nc.tensor.matmul(p1[:], t_wp, s_all[:])
            h1 = smp.tile([HID, BPC], f32, tag="h1")
            nc.vector._custom_dve(
                LRELU2X, out=h1[:], in0=p1[:], in1=t_bp4, s0=SLOPE
            )
            p2 = pp.tile([HID, BPC], f32, tag="mm")
            nc.tensor.matmul(p2[:], t_wo, h1[:])
            fin = smp.tile([HID, BPC], f32, tag="fin")
            nc.vector._custom_dve(
                LRELU2X, out=fin[:], in0=p2[:], in1=t_bo4, s0=SLOPE
            )
            nc.sync.dma_start(d_out[:], fin[:])

    nc.compile()
    return nc


def host_prep(inputs):
    """Host-side prep: per-batch gi/gjb (tiny matmuls) + packing."""
    x_img = np.asarray(inputs["x_img"], np.float32)
    W1 = np.asarray(inputs["W1"], np.float32)
    b1 = np.asarray(inputs["b1"], np.float32)
    W2 = np.asarray(inputs["W2"], np.float32)
    b2 = np.asarray(inputs["b2"], np.float32)
    Wp = np.asarray(inputs["Wp"], np.float32)
    bp = np.asarray(inputs["bp"], np.float32)
    Wo = np.asarray(inputs["Wo"], np.float32)
    bo = np.asarray(inputs["bo"], np.float32)

    x = x_img.reshape(B, C, L)  # [b, c, l]
    coords = np.arange(L, dtype=np.float32)
    Wa, Wb = W1[:C], W1[C + 1 : C + 1 + C]          # [128, 64] each
    GaT = coords[:, None] * W1[C][None, :]           # [144, 64]
    GbT = coords[:, None] * W1[C + 1 + C][None, :] + b1[None, :]

    # gi[b] = x[b].T @ Wa + GaT -> [144, 64]; stored [64, 144]
    gi = np.einsum("bcl,ch->bhl", x, Wa) + GaT.T[None]   # [B, 64, 144]
    gjb = np.einsum("bcl,ch->bhl", x, Wb) + GbT.T[None]  # [B, 64, 144]
    gi16 = gi.astype(np.float16)
    gjb16 = gjb.astype(np.float16)

    w2d = np.zeros((128, 128), np.float16)
    w2d[0:64, 0:64] = W2.astype(np.float16)
    w2d[64:128, 64:128] = W2.astype(np.float16)

    c32 = np.zeros((128, _C32_COLS), np.float32)
    c32[:, _C_B2C] = np.tile(b2, 2)
    c32[0:HID, _C_WP : _C_WP + HID] = Wp
    c32[0:HID, _C_WO : _C_WO + HID] = Wo
    c32[0:HID, _C_BP4 : _C_BP4 + BPC] = np.repeat(bp[:, None], BPC, axis=1)
    c32[0:HID, _C_BO4 : _C_BO4 + BPC] = np.repeat(bo[:, None], BPC, axis=1)

    base = {"w2d": np.ascontiguousarray(w2d), "c32": np.ascontiguousarray(c32)}
    in_maps = []
    for k in range(NCORES):
        bs = [BPC * k + i for i in range(BPC)]
        gid32 = np.zeros((2, 128, 32 * L), np.float16)
        gid16 = np.zeros((2, 128, 16 * L), np.float16)
        gjbp = np.zeros((2, 128, L), np.float16)
        for g in range(2):
            for h in range(2):
                bb = bs[2 * g + h]
                r = slice(64 * h, 64 * h + 64)
                gid32[g, r] = np.repeat(gi16[bb], 32, axis=1)
                gid16[g, r] = np.repeat(gi16[bb], 16, axis=1)
                gjbp[g, r] = gjb16[bb]
        m = dict(base)
        m["gid32"] = np.ascontiguousarray(gid32)
        m["gid16"] = np.ascontiguousarray(gid16)
        m["gjb"] = np.ascontiguousarray(gjbp)
        in_maps.append(m)
    return in_maps


def kernel(**inputs) -> np.ndarray:
    from concourse.bass_utils import run_bass_kernel_spmd

    if "nc" not in _cache:
        _cache["nc"] = build_nc()
    nc = _cache["nc"]
    in_maps = host_prep(inputs)
    res = run_bass_kernel_spmd(nc, in_maps, core_ids=list(range(NCORES)))
    out = np.concatenate([r["out"].T for r in res.results], axis=0)  # [32, 64]
    return np.ascontiguousarray(out, np.float32)
